# revision 84
# baseline (speedup 1.0000x reference)
"""CAMoE GNN layer (GCNConv experts x3, softmax gating) on 8 Trainium2 cores.

Sharding (per hint): nodes sharded across cores by TARGET; edges partitioned
by target node so the segment-sum stays core-local. Host does integer/index
preprocessing plus O(N*D) input prescaling; the O(E*D) work runs on device.

Math: agg_i = A_hat @ (x @ W_i) + b_i with A_hat = D^-1/2 (A+I) D^-1/2.
Aggregation is linear, so aggregate once on inputs: z = A_hat @ x, then
agg_i = z @ W_i + b_i. Host prescales xs = x * dinv (folds the source-side
norm into the gathered rows); the target-side dinv is applied on device as a
per-partition scale inside the ReLU activation. Self-loop terms never touch
the edge stream: they enter as one extra PE matmul per bin from a dense
slot-ordered xs tile.

Device pipeline per core (bin-major):
  - Target nodes are LPT-balanced into 98 bins x 128 slots per core. Per bin,
    non-self edges are sorted by source and packed greedily into full 128-edge
    chunks; each chunk is assigned one of 4 overlapping 32768-row source
    windows (int16 gather-index limit), so padding is only the final partial
    chunk per bin (~1%) plus deal slack.
  - Per (group-of-8-bins, window): one SWDGE dma_gather batch-fetches xs
    rows (bf16, pair-duplicated to satisfy the 256B descriptor minimum).
  - Per (bin, window): ONE DVE is_equal builds the one-hot scatter matrix for
    all cj chunks at once in a chunk-innermost layout [128e, 128t, cj] —
    every operand keeps a packed 2-byte last dim, so the DVE 2x mode stays
    active.
  - Per chunk: PE matmul zT_psum[f,t] += msg[e,f]-contract-S[e,t],
    accumulating one PSUM tile across all windows of the bin (z produced
    transposed: no on-chip transpose needed for the expert matmuls).
  - Per bin (fused phase 2): ACT copies zT to SBUF bf16; PE computes all 3
    expert outputs in one [64,192] matmul plus the self-loop term from the
    xs-by-slot tile; ACT applies ReLU with per-slot dinv scale; PE computes
    gating logits (K=4); ACT exp with free accum_out running sum; DVE
    reciprocal + two fused scalar_tensor_tensor ops combine the gate-weighted
    experts; ACT writes the normalized fp32 output column.
"""

import numpy as np

N = 100000
E = 1600000
D = 64
NEXP = 3
GC = 4
TEMP = 101.0
NCORES = 8
P = 128
TILES = 98                  # target bins per core
SLOTS = TILES * P           # 12544 output rows per core (>= 12500)
NBINS = NCORES * TILES      # 784 global target bins
CHUNK_SRC = 32768           # int16 gather-index limit (window width)
NW = 4                      # source windows
WSTARTS = (0, 25000, 50000, N - CHUNK_SRC)
SC = 64                     # max 128-edge chunks per dma_gather call
GBINS = 8                   # bins per processing group
MSG_BUFS = 3                # msg tiles in flight per window pool
PREFETCH = 2                # groups of gathers issued ahead of processing
IDXPF = 0                   # extra idx-copy group lookahead beyond PREFETCH

F32 = np.float32


def _bf16(a):
    import ml_dtypes
    return np.asarray(a, dtype=ml_dtypes.bfloat16)


def _host_prep(edge_index):
    """Pure index preprocessing: bin balancing, per-core edge-pair matching
    (two plain bf16 rows share one 256B gather descriptor), per-bin
    source-sorted chunk packing with window assignment for the unpaired
    remainder, signature-matched deal, per-core stream layout."""
    src_e = edge_index[0].astype(np.int64)
    tgt_e = edge_index[1].astype(np.int64)
    deg = np.bincount(tgt_e, minlength=N) + 1     # in-degree incl. self loop
    dinv = (1.0 / np.sqrt(deg.astype(np.float64))).astype(F32)

    # --- balance target nodes into NBINS bins of <=128 slots (LPT by degree) ---
    import heapq
    order = np.argsort(-deg, kind="stable")
    heap = [(0, b) for b in range(NBINS)]
    heapq.heapify(heap)
    counts = np.zeros(NBINS, np.int64)
    node_bin = np.empty(N, np.int32)
    node_slot = np.empty(N, np.int32)
    deg_l = deg.tolist()
    for n in order.tolist():
        while True:
            load, b = heapq.heappop(heap)
            if counts[b] < P:
                break
        node_bin[n] = b
        node_slot[n] = counts[b]
        counts[b] += 1
        heapq.heappush(heap, (load + deg_l[n], b))

    # --- per-bin edge lists sorted by source ---
    ebin = node_bin[tgt_e].astype(np.int64)
    eorder = np.lexsort((src_e, ebin))
    bcnt = np.bincount(ebin, minlength=NBINS)
    bstart = np.zeros(NBINS + 1, np.int64)
    np.cumsum(bcnt, out=bstart[1:])
    WS = np.asarray(WSTARTS, np.int64)
    WE = WS + CHUNK_SRC

    # --- deal signature from a pure-single greedy pack (pre-matching proxy) ---
    C0 = np.zeros((NBINS, NW), np.int64)
    for b in range(NBINS):
        el = eorder[bstart[b]:bstart[b + 1]]
        ss = src_e[el]
        n = len(el)
        i = 0
        while i < n:
            w = int(np.searchsorted(WS, ss[i], side="right")) - 1
            lim = int(np.searchsorted(ss, WE[w]))
            j = min(i + P, lim)
            C0[b, w] += 1
            i = j
    sort_idx = np.lexsort((C0[:, 3], C0[:, 2], C0[:, 1], C0[:, 0]))[::-1]
    bin_of = sort_idx.reshape(TILES, NCORES)     # [pos, core] -> bin

    # --- per-core matching + single-stream packing ---
    # W streams: 0..NW-1 = source windows (singles, pair-dup rows),
    #            NW = pair table (one 256B slot = two matched rows)
    PAIRW = NW
    WTOT = NW + 1
    PCAP = 32640                                  # pair slots (int16 idx head)
    Ck = np.zeros((NCORES, TILES, WTOT), np.int64)
    core_bins = []   # per core: per pos: dict(pairsA, pairsB, schunks=[(w, el)])
    # match in a strided position order so pair-rich (match-early) and
    # pair-poor (match-late) bins mix within every processing group
    morder = [(j * 37) % TILES for j in range(TILES)]
    for k in range(NCORES):
        seen = np.zeros(N, bool)
        nslots = 0
        binsk = [None] * TILES
        for j in morder:
            b = bin_of[j, k]
            el = eorder[bstart[b]:bstart[b + 1]]
            ss = src_e[el]
            u, fi = np.unique(ss, return_index=True)
            new = ~seen[u]
            seen[u] = True
            cand = fi[new]                        # sorted positions in el
            npair = min(len(cand) // 2, PCAP - nslots, 5 * P)
            a_pos = cand[0:2 * npair:2]
            b_pos = cand[1:2 * npair:2]
            nslots += npair
            pm = np.zeros(len(el), bool)
            pm[a_pos] = True
            pm[b_pos] = True
            sing = el[~pm]                        # still sorted by src
            sss = src_e[sing]
            schunks = []
            i = 0
            n = len(sing)
            while i < n:
                w = int(np.searchsorted(WS, sss[i], side="right")) - 1
                lim = int(np.searchsorted(sss, WE[w]))
                jj = min(i + P, lim)
                schunks.append((w, sing[i:jj]))
                Ck[k, j, w] += 1
                i = jj
            Ck[k, j, PAIRW] = -(-npair // P) if npair else 0
            binsk[j] = dict(pA=el[a_pos], pB=el[b_pos], schunks=schunks)
        core_bins.append(binsk)
    C_used = Ck.max(axis=0)                       # [pos, WTOT]

    # --- bin-position groups: full groups, small tail groups (cheap drain) ---
    group_sizes = []
    left = TILES
    while left > 10:
        group_sizes.append(GBINS)
        left -= GBINS
    while left > 0:
        s = min(4, left)
        group_sizes.append(s)
        left -= s
    groups = []
    o = 0
    for s in group_sizes:
        groups.append(list(range(o, o + s)))
        o += s
    ngroups = len(groups)

    # stream order: (group, stream w, pos-in-group, chunk).
    # cols per chunk: 1 for single streams, 2 for the pair stream (A/B planes)
    CPC = [1] * NW + [2]
    col_ofs = np.zeros((TILES, WTOT), np.int64)   # tgt_enc column index
    ww_ofs = np.zeros((TILES, WTOT), np.int64)    # within-stream chunk index
    sec_chunks = np.zeros((ngroups, WTOT), np.int64)
    ccols = 0
    wwp = np.zeros(WTOT, np.int64)
    for g, grp in enumerate(groups):
        for w in range(WTOT):
            for j in grp:
                cj = int(C_used[j, w])
                col_ofs[j, w] = ccols
                ww_ofs[j, w] = wwp[w]
                ccols += cj * CPC[w]
                wwp[w] += cj
                sec_chunks[g, w] += cj
    NCOLS = ccols
    M_w = wwp                                     # chunks per stream
    M_tot = int(M_w.sum())

    # gather calls: per (group, stream), split into <=SC-chunk windows
    calls = []  # (w, ww_start, n_chunks, group, local_ofs)
    for g in range(ngroups):
        for w in range(WTOT):
            n = int(sec_chunks[g, w])
            if n == 0:
                continue
            j0 = groups[g][0]
            start = int(ww_ofs[j0, w])
            done = 0
            while done < n:
                ck = min(SC, n - done)
                calls.append((w, start + done, ck, g, done))
                done += ck

    # --- per-core padded streams ---
    cores = []
    for k in range(NCORES):
        senc = np.full(NCOLS * P, -1.0, F32)      # slot id per col-lane
        ls = [np.zeros(int(M_w[w]) * P, np.int64) for w in range(WTOT)]
        nslots = 0
        for j in range(TILES):
            bk = core_bins[k][j]
            wcc = np.zeros(WTOT, np.int64)
            for (w, el) in bk["schunks"]:
                cc = int(wcc[w])
                wcc[w] += 1
                cnt = len(el)
                ls[w][(int(ww_ofs[j, w]) + cc) * P + np.arange(cnt)] = \
                    src_e[el] - WS[w]
                senc[(int(col_ofs[j, w]) + cc) * P + np.arange(cnt)] = \
                    node_slot[tgt_e[el]]
            pA, pB = bk["pA"], bk["pB"]
            npair = len(pA)
            for c in range(int(-(-npair // P)) if npair else 0):
                lo_ = c * P
                hi = min(npair, lo_ + P)
                cnt = hi - lo_
                ls[PAIRW][(int(ww_ofs[j, PAIRW]) + c) * P + np.arange(cnt)] = \
                    nslots + lo_ + np.arange(cnt)
                base = (int(col_ofs[j, PAIRW]) + 2 * c) * P
                senc[base + np.arange(cnt)] = node_slot[tgt_e[pA[lo_:hi]]]
                senc[base + P + np.arange(cnt)] = node_slot[tgt_e[pB[lo_:hi]]]
            nslots += npair
        gidx = []
        for w in range(WTOT):
            Lw = int(M_w[w]) * P
            arr = ls[w].astype(np.int16).reshape(Lw // 16, 16).T
            gidx.append(np.ascontiguousarray(np.tile(arr, (8, 1))))
        tgt_enc = np.ascontiguousarray(_bf16(senc.reshape(NCOLS, P).T))
        pairsA = np.concatenate([core_bins[k][j]["pA"] for j in range(TILES)])
        pairsB = np.concatenate([core_bins[k][j]["pB"] for j in range(TILES)])
        cores.append(dict(gidx=gidx, tgt_enc=tgt_enc,
                          pairsA=src_e[pairsA], pairsB=src_e[pairsB]))

    return dict(
        dinv=dinv, node_bin=node_bin, node_slot=node_slot, bin_of=bin_of,
        C_used=C_used, M_w=M_w, M_tot=M_tot, NCOLS=NCOLS, CPC=CPC,
        cores=cores, groups=groups, col_ofs=col_ofs, ww_ofs=ww_ofs,
        sec_chunks=sec_chunks, calls=calls,
    )


def _core_tensors(prep, k, xs, xs2_parts, gate_features, W, b, Wg, consts):
    """Build the in_map (name -> np.ndarray) for core k."""
    c = prep["cores"][k]
    node_bin, node_slot = prep["node_bin"], prep["node_slot"]
    bin_of, dinv = prep["bin_of"], prep["dinv"]

    dinvt = np.zeros((P, TILES), F32)
    gft = np.zeros((GC, SLOTS), F32)
    xo = np.zeros((SLOTS, D), F32)
    for j in range(TILES):
        bnodes = np.nonzero(node_bin == bin_of[j, k])[0]
        sl = node_slot[bnodes]
        dinvt[sl, j] = dinv[bnodes]
        gft[:, j * P + sl] = np.asarray(gate_features)[bnodes].T
        xo[j * P + sl] = xs[bnodes]

    npair = len(c["pairsA"])
    xpair = np.zeros((32768, 2 * D), F32)
    xpair[:npair, :D] = xs[c["pairsA"]]
    xpair[:npair, D:] = xs[c["pairsB"]]

    m = dict(
        tgt_enc=c["tgt_enc"], dinvt=dinvt, gft=np.ascontiguousarray(_bf16(gft)),
        xot=np.ascontiguousarray(_bf16(xo.T)),
        xpair=np.ascontiguousarray(_bf16(xpair)),
        wcat=np.ascontiguousarray(_bf16(
            np.concatenate([W[i] for i in range(NEXP)], axis=1))),
        wg=np.ascontiguousarray(_bf16(Wg)),
        **consts,
    )
    for w in range(NW + 1):
        m[f"gidx{w}"] = c["gidx"][w]
    for w, xp in enumerate(xs2_parts):
        m[f"x{w}"] = xp
    return m


def _build_program(prep, with_bias, debug_taps=False):
    """Build the Bass/Tile program. Only uses chunk counts / layout metadata
    (identical across cores), never float data."""
    import concourse.bass as bass
    import concourse.tile as tile
    from concourse import bacc, mybir

    dt = mybir.dt
    nc = bacc.Bacc("TRN2", target_bir_lowering=False, debug=False,
                   enable_asserts=False, num_devices=NCORES)

    C_used = prep["C_used"]
    M_w = prep["M_w"]
    NCOLS = prep["NCOLS"]
    CPC = prep["CPC"]
    groups = prep["groups"]
    col_ofs = prep["col_ofs"]
    ww_ofs = prep["ww_ofs"]
    sec_chunks = prep["sec_chunks"]
    calls = prep["calls"]
    ngroups = len(groups)
    WTOT = NW + 1
    CJMAX = max(int(C_used[:, w].max()) * CPC[w] for w in range(WTOT))

    x_d = [nc.dram_tensor(f"x{w}", [CHUNK_SRC, 2 * D], dt.bfloat16, kind="ExternalInput").ap()
           for w in range(NW)]
    x_d.append(nc.dram_tensor("xpair", [32768, 2 * D], dt.bfloat16,
                              kind="ExternalInput").ap())
    gidx_d = [nc.dram_tensor(f"gidx{w}", [P, int(M_w[w]) * 8], dt.int16,
                             kind="ExternalInput").ap() for w in range(WTOT)]
    tgt_d = nc.dram_tensor("tgt_enc", [P, NCOLS], dt.bfloat16, kind="ExternalInput").ap()
    dinvt_d = nc.dram_tensor("dinvt", [P, TILES], dt.float32, kind="ExternalInput").ap()
    gft_d = nc.dram_tensor("gft", [GC, SLOTS], dt.bfloat16, kind="ExternalInput").ap()
    xot_d = nc.dram_tensor("xot", [D, SLOTS], dt.bfloat16, kind="ExternalInput").ap()
    wcat_d = nc.dram_tensor("wcat", [D, NEXP * D], dt.bfloat16, kind="ExternalInput").ap()
    wg_d = nc.dram_tensor("wg", [GC, NEXP], dt.bfloat16, kind="ExternalInput").ap()
    iota2_d = nc.dram_tensor("iota2", [P, P * CJMAX], dt.bfloat16, kind="ExternalInput").ap()
    if with_bias:
        bbc_d = nc.dram_tensor("bbc", [P, NEXP * D], dt.float32, kind="ExternalInput").ap()
    out_d = nc.dram_tensor("out", [P, TILES * D], dt.bfloat16, kind="ExternalOutput").ap()
    if debug_taps:
        zdbg_d = nc.dram_tensor("zdbg", [D, SLOTS], dt.float32, kind="ExternalOutput").ap()

    calls_by_g = [[] for _ in range(ngroups)]
    for (w, wws, ck, g, lofs) in calls:
        calls_by_g[g].append((w, wws, ck, lofs))

    with tile.TileContext(nc) as tc:
        with tc.tile_pool(name="const", bufs=1) as cpool, \
             tc.tile_pool(name="m0", bufs=MSG_BUFS) as mp0, \
             tc.tile_pool(name="m1", bufs=MSG_BUFS) as mp1, \
             tc.tile_pool(name="m2", bufs=MSG_BUFS) as mp2, \
             tc.tile_pool(name="m3", bufs=MSG_BUFS) as mp3, \
             tc.tile_pool(name="m4", bufs=2) as mp4, \
             tc.tile_pool(name="idx", bufs=4 * (PREFETCH + IDXPF) + 2) as tpool, \
             tc.tile_pool(name="sel", bufs=4) as spool, \
             tc.tile_pool(name="ph2", bufs=4) as kpool, \
             tc.tile_pool(name="outp", bufs=1) as opool, \
             tc.tile_pool(name="pz", bufs=5, space="PSUM") as pz, \
             tc.tile_pool(name="ph", bufs=2, space="PSUM") as ph, \
             tc.tile_pool(name="py", bufs=1, space="PSUM") as py:
            mpools = [mp0, mp1, mp2, mp3, mp4]

            def load_const(ap_d, shape, tag, dtype=dt.float32):
                t = cpool.tile(shape, dtype, tag=tag, name=tag)
                nc.scalar.dma_start(t[:], ap_d)
                return t

            iota2_sb = load_const(iota2_d, [P, P * CJMAX], tag="iota2", dtype=dt.bfloat16)
            tgt_sb = load_const(tgt_d, [P, NCOLS], tag="tgt", dtype=dt.bfloat16)
            dinvt_sb = load_const(dinvt_d, [P, TILES], tag="dinvt")
            gft_sb = load_const(gft_d, [GC, SLOTS], tag="gft", dtype=dt.bfloat16)
            xot_sb = load_const(xot_d, [D, SLOTS], tag="xot", dtype=dt.bfloat16)
            wcat_sb = load_const(wcat_d, [D, NEXP * D], tag="wcat", dtype=dt.bfloat16)
            wg_sb = load_const(wg_d, [GC, NEXP], tag="wg", dtype=dt.bfloat16)
            if with_bias:
                bbc_sb = load_const(bbc_d, [P, NEXP * D], tag="bbc")

            out_sb = opool.tile([P, TILES * D], dt.bfloat16)

            msg_tiles = {}
            idx_tiles = {}

            def issue_idx(g):
                for ci, (w, wws, ck, lofs) in enumerate(calls_by_g[g]):
                    idx_t = tpool.tile([P, SC * 8], dt.int16, tag="idx")
                    nc.sync.dma_start(idx_t[:, :ck * 8],
                                      gidx_d[w][:, wws * 8:(wws + ck) * 8])
                    idx_tiles[(g, ci)] = idx_t

            def issue_gathers(g):
                for w in range(WTOT):
                    if sec_chunks[g, w] == 0:
                        continue
                    msg_tiles[(g, w)] = mpools[w].tile(
                        [P, int(sec_chunks[g, w]) * 2 * D], dt.bfloat16,
                        tag=f"msg{w}", name=f"msg{w}")
                for ci, (w, wws, ck, lofs) in enumerate(calls_by_g[g]):
                    idx_t = idx_tiles.pop((g, ci))
                    mt = msg_tiles[(g, w)]
                    nc.gpsimd.dma_gather(
                        out_ap=mt[:, lofs * 2 * D:(lofs + ck) * 2 * D]
                        .rearrange("p (c f) -> p c f", f=2 * D),
                        in_ap=x_d[w],
                        idxs_ap=idx_t[:, :ck * 8],
                        num_idxs=ck * P,
                        num_idxs_reg=ck * P,
                        elem_size=2 * D,
                        single_packet=False,
                    )

            def process_group(g):
                for j in groups[g]:
                    zT_ps = pz.tile([D, P], dt.float32, tag="zT")
                    total_mm = int(sum(int(C_used[j, w]) * CPC[w]
                                       for w in range(WTOT)))
                    done = 0
                    for w in range(WTOT):
                        cj = int(C_used[j, w])
                        if cj == 0:
                            continue
                        ncol = cj * CPC[w]
                        co = int(col_ofs[j, w])
                        lo = int(ww_ofs[j, w] - ww_ofs[groups[g][0], w])
                        S2 = spool.tile([P, P * ncol], dt.bfloat16, tag="S2")
                        if ncol == 1:
                            nc.vector.tensor_tensor(
                                out=S2[:],
                                in0=tgt_sb[:, co:co + 1].to_broadcast([P, P]),
                                in1=iota2_sb[:].rearrange("p (t c) -> p t c", c=CJMAX)[:, :, 0],
                                op=mybir.AluOpType.is_equal,
                            )
                        else:
                            nc.vector.tensor_tensor(
                                out=S2[:].rearrange("p (t c) -> p t c", c=ncol),
                                in0=tgt_sb[:, co:co + ncol].unsqueeze(1)
                                .to_broadcast([P, P, ncol]),
                                in1=iota2_sb[:].rearrange("p (t c) -> p t c", c=CJMAX)
                                [:, :, :ncol],
                                op=mybir.AluOpType.is_equal,
                            )
                        mt = msg_tiles[(g, w)]
                        for c in range(cj):
                            for h in range(CPC[w]):
                                o = (lo + c) * 2 * D + h * D
                                col = c * CPC[w] + h
                                nc.tensor.matmul(
                                    out=zT_ps[:],
                                    lhsT=mt[:, o:o + D],
                                    rhs=S2[:].rearrange("p (t c) -> p t c", c=ncol)[:, :, col]
                                    if ncol > 1 else S2[:],
                                    start=(done == 0), stop=(done == total_mm - 1),
                                )
                                done += 1
                    # ---- fused phase 2 for bin j ----
                    zT = kpool.tile([D, P], dt.bfloat16, tag="zTs")
                    nc.scalar.copy(zT[:], zT_ps[:])
                    if debug_taps:
                        zf = kpool.tile([D, P], dt.float32, tag="zf")
                        nc.vector.tensor_copy(zf[:], zT_ps[:])
                        nc.sync.dma_start(zdbg_d[:, j * P:(j + 1) * P], zf[:])
                    h_ps = ph.tile([P, NEXP * D], dt.float32, tag="h")
                    nc.tensor.matmul(out=h_ps[:], lhsT=zT[:], rhs=wcat_sb[:],
                                     start=True, stop=False)
                    nc.tensor.matmul(out=h_ps[:], lhsT=xot_sb[:, j * P:(j + 1) * P],
                                     rhs=wcat_sb[:], start=False, stop=True)
                    h = kpool.tile([P, NEXP * D], dt.bfloat16, tag="hs")
                    if with_bias:
                        hb = kpool.tile([P, NEXP * D], dt.float32, tag="hb")
                        nc.scalar.mul(hb[:], h_ps[:], mul=dinvt_sb[:, j:j + 1])
                        nc.vector.tensor_add(hb[:], hb[:], bbc_sb[:])
                        nc.scalar.activation(h[:], hb[:],
                                             mybir.ActivationFunctionType.Relu)
                    else:
                        nc.scalar.activation(h[:], h_ps[:],
                                             mybir.ActivationFunctionType.Relu,
                                             scale=dinvt_sb[:, j:j + 1])
                    y_ps = py.tile([P, NEXP], dt.float32, tag="y")
                    nc.tensor.matmul(out=y_ps[:], lhsT=gft_sb[:, j * P:(j + 1) * P],
                                     rhs=wg_sb[:], start=True, stop=True)
                    ge = kpool.tile([P, NEXP], dt.float32, tag="ge")
                    gs = kpool.tile([P, 1], dt.float32, tag="gs")
                    nc.scalar.activation(ge[:], y_ps[:],
                                         mybir.ActivationFunctionType.Exp,
                                         scale=float(1.0 / TEMP), accum_out=gs[:])
                    gr = kpool.tile([P, 1], dt.float32, tag="gr")
                    nc.vector.reciprocal(gr[:], gs[:])
                    acc0 = kpool.tile([P, D], dt.bfloat16, tag="a0")
                    nc.scalar.mul(acc0[:], h[:, 0:D], mul=ge[:, 0:1])
                    acc1 = kpool.tile([P, D], dt.bfloat16, tag="a1")
                    nc.vector.scalar_tensor_tensor(
                        out=acc1[:], in0=h[:, D:2 * D], scalar=ge[:, 1:2],
                        in1=acc0[:], op0=mybir.AluOpType.mult,
                        op1=mybir.AluOpType.add)
                    acc2 = kpool.tile([P, D], dt.bfloat16, tag="a2")
                    nc.vector.scalar_tensor_tensor(
                        out=acc2[:], in0=h[:, 2 * D:3 * D], scalar=ge[:, 2:3],
                        in1=acc1[:], op0=mybir.AluOpType.mult,
                        op1=mybir.AluOpType.add)
                    nc.scalar.mul(out_sb[:, j * D:(j + 1) * D], acc2[:],
                                  mul=gr[:])

            for g0 in range(min(PREFETCH + IDXPF, ngroups)):
                issue_idx(g0)
            for g0 in range(min(PREFETCH, ngroups)):
                issue_gathers(g0)
            for g in range(ngroups):
                if g + PREFETCH + IDXPF < ngroups:
                    issue_idx(g + PREFETCH + IDXPF)
                if g + PREFETCH < ngroups:
                    issue_gathers(g + PREFETCH)
                process_group(g)
                j0, j1 = groups[g][0], groups[g][-1]
                nc.scalar.dma_start(out_d[:, j0 * D:(j1 + 1) * D],
                                    out_sb[:, j0 * D:(j1 + 1) * D])
    nc.finalize()
    return nc


def kernel(x, edge_index, gate_features, W, b, Wg):
    from concourse.bass_utils import run_bass_kernel_spmd

    x = np.asarray(x, dtype=F32)
    edge_index = np.asarray(edge_index)
    W = np.asarray(W, dtype=F32)
    b = np.asarray(b, dtype=F32)
    Wg = np.asarray(Wg, dtype=F32)
    gate_features = np.asarray(gate_features, dtype=F32)

    prep = _host_prep(edge_index)
    with_bias = bool(np.any(b != 0))

    # prescale by source-side dinv, pair-duplicate rows (256B descriptors)
    xs = x * prep["dinv"][:, None]
    xs2 = _bf16(np.concatenate([xs, xs], axis=1))  # [N, 128]
    xs2_parts = [np.ascontiguousarray(xs2[ws:ws + CHUNK_SRC]) for ws in WSTARTS]

    CJMAX = max(int(prep["C_used"][:, w].max()) * prep["CPC"][w]
            for w in range(len(prep["CPC"])))
    iota2 = np.broadcast_to(np.arange(P, dtype=F32)[None, :, None],
                            (P, P, CJMAX)).reshape(P, P * CJMAX)
    consts = dict(iota2=np.ascontiguousarray(_bf16(iota2)))
    if with_bias:
        consts["bbc"] = np.ascontiguousarray(np.broadcast_to(
            np.concatenate([b[i] for i in range(NEXP)])[None, :],
            (P, NEXP * D)).astype(F32))

    in_maps = [_core_tensors(prep, k, xs, xs2_parts, gate_features, W, b, Wg, consts)
               for k in range(NCORES)]
    nc = _build_program(prep, with_bias)
    res = run_bass_kernel_spmd(nc, in_maps, core_ids=list(range(NCORES)))
    global LAST_RESULTS, LAST_NC
    LAST_RESULTS = res
    LAST_NC = nc

    node_bin, node_slot = prep["node_bin"], prep["node_slot"]
    bin_of = prep["bin_of"]
    bin_core = np.empty(NBINS, np.int64)
    bin_pos = np.empty(NBINS, np.int64)
    for j in range(TILES):
        for k in range(NCORES):
            bin_core[bin_of[j, k]] = k
            bin_pos[bin_of[j, k]] = j
    out = np.empty((N, D), F32)
    # out tensor is [P(slot), TILES*D] bf16: node n at [slot, pos*D:(pos+1)*D]
    per_core = np.stack([np.asarray(res.results[k]["out"], dtype=F32)
                         for k in range(NCORES)])
    per_core = per_core.reshape(NCORES, P, TILES, D).transpose(0, 2, 1, 3)
    out[:] = per_core[bin_core[node_bin], bin_pos[node_bin], node_slot]
    return out
